# revision 1
# baseline (speedup 1.0000x reference)
"""MeanAggregator (GraphSAGE-style) Bass kernel for Trainium2, 8 NeuronCores.

Reference semantics (per output row r):
    samp = [to_neighs[r, :], nodes[r]]              # 33 ids
    w[k] = 1 if samp[k] is the first occurrence of its value in the row else 0
    out[r] = (sum_k w[k] * features[samp[k]]) / sum_k w[k]

Distribution: data-parallel over the 50000-row batch; 6250 rows per core,
features table replicated on all 8 cores.

Per-core device kernel:
  - rows are mapped partition-major: global row g = p*49 + t  (p=partition,
    t=tile index 0..48), 49 tiles of 128 rows (6272 rows incl. padding)
  - dedup weights computed in one batched pass (delta-shift equality compares)
  - per tile: 33 indirect-DMA gathers (128 rows x 512B each), then 33
    accumulating matmuls with diagonal weight matrices, then scale by 1/cnt
"""
import numpy as np

N = 50000
K = 32
KP1 = K + 1          # 33
V = 500000
D = 128
NCORES = 8
P = 128
ROWS_PER_CORE = N // NCORES          # 6250
TILES = (ROWS_PER_CORE + P - 1) // P # 49
ROWS_PAD = TILES * P                 # 6272

_cached = {}


def _build(rep=1):
    """rep>1 repeats the whole compute body in-kernel (for timing only)."""
    import concourse.bass as bass
    import concourse.bacc as bacc
    import concourse.mybir as mybir
    import concourse.tile as tile
    from concourse.masks import make_identity

    nc = bacc.Bacc("TRN2", target_bir_lowering=False, debug=False)
    f32 = mybir.dt.float32
    i32 = mybir.dt.int32

    samp = nc.dram_tensor("samp", [ROWS_PAD, KP1], i32, kind="ExternalInput").ap()
    feat = nc.dram_tensor("feat", [V, D], f32, kind="ExternalInput").ap()
    out = nc.dram_tensor("out", [ROWS_PAD, D], f32, kind="ExternalOutput").ap()

    L = TILES * KP1  # 1617 ids per partition

    with tile.TileContext(nc) as tc:
        with tc.tile_pool(name="idx", bufs=1) as ipool, \
             tc.tile_pool(name="w", bufs=1) as wpool, \
             tc.tile_pool(name="tmp", bufs=2) as tpool, \
             tc.tile_pool(name="emb", bufs=3) as epool, \
             tc.tile_pool(name="diag", bufs=2) as dpool, \
             tc.tile_pool(name="o", bufs=3) as opool, \
             tc.tile_pool(name="psum", bufs=2, space="PSUM") as ppool:

            # ---- load ids: partition p holds rows [p*49, (p+1)*49) ----
            s_i = ipool.tile([P, L], i32)
            nc.sync.dma_start(
                out=s_i[:], in_=samp.rearrange("(p t) k -> p (t k)", p=P))

            identity = ipool.tile([P, P], f32)
            make_identity(nc, identity[:])

            for _r in range(rep):
                # ---- dedup weights (batched over all tiles) ----
                s_f = wpool.tile([P, L], f32)
                nc.vector.tensor_copy(s_f[:], s_i[:])           # int -> f32 (exact, <2^24)
                s3 = s_f[:].rearrange("p (t k) -> p t k", k=KP1)

                dc = wpool.tile([P, L], f32)
                nc.vector.memset(dc[:], 0.0)
                dc3 = dc[:].rearrange("p (t k) -> p t k", k=KP1)
                for delta in range(1, KP1):
                    eq = tpool.tile([P, TILES * (KP1 - delta)], f32, tag="eq")
                    eq3 = eq[:].rearrange("p (t k) -> p t k", k=KP1 - delta)
                    nc.vector.tensor_tensor(
                        out=eq3, in0=s3[:, :, delta:], in1=s3[:, :, :KP1 - delta],
                        op=mybir.AluOpType.is_equal)
                    nc.vector.tensor_tensor(
                        out=dc3[:, :, delta:], in0=dc3[:, :, delta:], in1=eq3,
                        op=mybir.AluOpType.add)

                w = wpool.tile([P, L], f32)
                nc.vector.tensor_scalar(
                    out=w[:], in0=dc[:], scalar1=0.0, scalar2=None,
                    op0=mybir.AluOpType.is_equal)
                w3 = w[:].rearrange("p (t k) -> p t k", k=KP1)

                cnt = wpool.tile([P, TILES], f32)
                nc.vector.tensor_reduce(
                    out=cnt[:], in_=w3, axis=mybir.AxisListType.X,
                    op=mybir.AluOpType.add)
                inv_cnt = wpool.tile([P, TILES], f32)
                nc.vector.reciprocal(inv_cnt[:], cnt[:])

                id_b = identity[:].rearrange("p (o f) -> p o f", o=1) \
                                  .to_broadcast((P, KP1, P))

                # ---- per-tile gather + weighted sum ----
                for t in range(TILES):
                    E = epool.tile([P, KP1 * D], f32)
                    for k in range(KP1):
                        nc.gpsimd.indirect_dma_start(
                            out=E[:, k * D:(k + 1) * D],
                            out_offset=None,
                            in_=feat[:],
                            in_offset=bass.IndirectOffsetOnAxis(
                                ap=s_i[:, t * KP1 + k:t * KP1 + k + 1], axis=0))

                    Dg = dpool.tile([P, KP1 * P], f32)
                    wb = w3[:, t, :].rearrange("p (k o) -> p k o", o=1) \
                                    .to_broadcast((P, KP1, P))
                    nc.vector.tensor_tensor(
                        out=Dg[:].rearrange("p (k f) -> p k f", k=KP1),
                        in0=id_b, in1=wb, op=mybir.AluOpType.mult)

                    acc = ppool.tile([P, D], f32, space="PSUM")
                    for k in range(KP1):
                        nc.tensor.matmul(
                            out=acc[:],
                            lhsT=Dg[:, k * P:(k + 1) * P],
                            rhs=E[:, k * D:(k + 1) * D],
                            start=(k == 0), stop=(k == KP1 - 1))

                    o = opool.tile([P, D], f32)
                    nc.vector.tensor_scalar(
                        out=o[:], in0=acc[:], scalar1=inv_cnt[:, t:t + 1],
                        scalar2=None, op0=mybir.AluOpType.mult)
                    nc.sync.dma_start(
                        out=out.rearrange("(p t) d -> p t d", t=TILES)[:, t, :],
                        in_=o[:])

    nc.compile()
    return nc


def _get_nc():
    if "nc" not in _cached:
        _cached["nc"] = _build()
    return _cached["nc"]


def kernel(nodes, to_neighs, features):
    from concourse.bass_utils import run_bass_kernel_spmd

    nodes = np.asarray(nodes).astype(np.int32, copy=False)
    to_neighs = np.asarray(to_neighs).astype(np.int32, copy=False)
    features = np.ascontiguousarray(np.asarray(features, dtype=np.float32))
    assert nodes.shape == (N,) and to_neighs.shape == (N, K)
    assert features.shape == (V, D)

    samp = np.concatenate([to_neighs, nodes[:, None]], axis=1)  # [N, 33]
    samp_pad = np.zeros((NCORES * ROWS_PAD, KP1), np.int32)
    for c in range(NCORES):
        samp_pad[c * ROWS_PAD:c * ROWS_PAD + ROWS_PER_CORE] = \
            samp[c * ROWS_PER_CORE:(c + 1) * ROWS_PER_CORE]

    nc = _get_nc()
    in_maps = [
        {"samp": samp_pad[c * ROWS_PAD:(c + 1) * ROWS_PAD], "feat": features}
        for c in range(NCORES)
    ]
    results = run_bass_kernel_spmd(nc, in_maps, list(range(NCORES))).results
    out = np.concatenate(
        [results[c]["out"][:ROWS_PER_CORE] for c in range(NCORES)], axis=0)
    return out

